# revision 30
# baseline (speedup 1.0000x reference)
"""Trainium2 kernel for nn_CompressedLinearRANS: out = x @ (w_int8*scale).T + bias.

Strategy: data-parallel over the 8192 rows of x (1024 rows per core, full
weight replicated per core). Per-core matmul M=1024, K=4096, N=4096 on the
PE array: the first 3584 of the 4096 contraction elements run in fp16, the
last 512 run as fp8(e4m3) DoubleRow matmuls (2x MAC rate). The fp8 share is
sized so the total quantization error (~1.5e-2 max-rel) stays safely inside
the 2e-2 tolerance.

All dtype conversion happens on the host: x/w are rounded to fp16/e4m3 (with
the per-tensor scale folded into w), so the device does only DMA + matmul +
a single fused bias-add/downcast eviction per output tile.

Host-prepped inputs per core:
  x16  [128, 28*1024] fp16  (fp16 part of x^T shard, k-chunk-major [p, ko, m])
  x8   [128, 4*1024]  e4m3  (fp8 part)
  w16  [8*128, 28*512] fp16 ((w.T*scale) fp16 part, slab-major [nt, p, ko, nf])
  w8   [8*128, 4*512]  e4m3 (fp8 part)
  bias_rep [128, 4096] fp32 (bias broadcast to 128 partitions)
Output per core: out [1024, 4096] fp16 ([p, mt, n] over partitions).
"""

import os

import numpy as np

import concourse.bass as bass
import concourse.mybir as mybir
import concourse.tile as tile
from concourse.bass_utils import run_bass_kernel_spmd

# ---------------------------------------------------------------------------
# Workaround for this walrus build: it rejects instructions carrying more
# than one sync wait command ("Too many sync wait commands",
# CoreV*GenImpl::setupSyncWait). After Tile lowering, split any instruction
# with >1 sem wait: keep the last wait on the instruction and hoist the rest
# onto same-engine NoOps inserted immediately before it (engines execute
# their stream in order, so this is semantics-preserving).
# ---------------------------------------------------------------------------


def _split_sync_waits(nc, max_waits=1):
    for fn in nc.m.functions:
        for bb in fn.blocks:
            out = []
            changed = False
            for inst in bb.instructions:
                si = inst.sync_info
                waits = list(si.on_wait) if si is not None and si.on_wait else []
                if len(waits) > max_waits:
                    changed = True
                    for w in waits[:-max_waits]:
                        nop = mybir.InstNoOp(
                            name=nc.get_next_instruction_name(),
                            sync_info=mybir.SyncInfo(on_wait=[w], on_update=[]),
                            bass_nofuse=True,
                            engine=inst.engine,
                        )
                        nc.register_instruction(nop)
                        out.append(nop)
                    inst.sync_info = mybir.SyncInfo(
                        on_wait=waits[-max_waits:],
                        on_update=list(si.on_update or []),
                    )
                out.append(inst)
            if changed:
                bb.instructions = out


# ---------------------------------------------------------------------------

P = 128
N_CORES = 8

OUT_F, IN_F = 4096, 4096
B, S = 4, 2048
M_TOTAL = B * S           # 8192 rows
M = M_TOTAL // N_CORES    # 1024 rows per core

KO = IN_F // P            # 32 k-chunks of 128
C8 = 4                    # k-chunks computed in fp8 DoubleRow
KF = KO - C8              # k-chunks computed in fp16
G8 = C8 // 2              # DoubleRow matmuls per output tile (2 chunks each)
MT = M // P               # 8 m-tiles of 128
NF = 512                  # moving free dim / PSUM bank width (fp32)
NT = OUT_F // NF          # 8 n-tiles

F32 = mybir.dt.float32
F16 = mybir.dt.float16
F8 = mybir.dt.float8e4
DR = mybir.MatmulPerfMode.DoubleRow

N_WARMUP_MM = 8           # PE clock warmup during the DMA lead-in (N=512 each)

# exposed for test.py
LAST_RESULTS = None


def build_nc(trn_type=None):
    """Build the per-core Bass module."""
    nc = bass.Bass() if trn_type is None else bass.Bass(trn_type)
    x16 = nc.dram_tensor("x16", [P, KF * M], F16, kind="ExternalInput")
    x8 = nc.dram_tensor("x8", [P, C8 * M], F8, kind="ExternalInput")
    w16 = nc.dram_tensor("w16", [NT * P, KF * NF], F16, kind="ExternalInput")
    w8 = nc.dram_tensor("w8", [NT * P, C8 * NF], F8, kind="ExternalInput")
    bias_rep = nc.dram_tensor("bias_rep", [P, OUT_F], F32, kind="ExternalInput")
    out = nc.dram_tensor("out", [M, OUT_F], F16, kind="ExternalOutput")

    x_view = x16.rearrange("p (ko m) -> p ko m", ko=KF)        # [P, KF, M]
    x8_view = x8.rearrange("p (c m) -> p c m", c=C8)           # [P, C8, M]
    w_view = w16.rearrange("(nt p) (ko nf) -> nt p ko nf", nt=NT, ko=KF)
    w8_view = w8.rearrange("(nt p) (c nf) -> nt p c nf", nt=NT, c=C8)
    out_view = out.rearrange("(mt p) n -> p mt n", p=P)        # [P, MT, N]

    with tile.TileContext(nc) as tc:
        with (
            tc.tile_pool(name="const", bufs=1) as const_pool,
            tc.tile_pool(name="xsb", bufs=1) as x_pool,
            tc.tile_pool(name="wsb", bufs=2) as w_pool,
            tc.tile_pool(name="w8sb", bufs=2) as w8_pool,
            tc.tile_pool(name="outsb", bufs=6) as out_pool,
            tc.tile_pool(name="warm", bufs=1) as warm_pool,
            tc.tile_pool(name="psum", bufs=8, space="PSUM") as psum_pool,
        ):
            # --- PE clock warmup: tiny self-contained matmuls on zeroed SBUF
            # during the initial DMA wait, so the HAM un-throttles before the
            # first real matmul. They write (start=True) into psum tiles that
            # the real accumulation groups reset anyway.
            warm = warm_pool.tile([2, NF], F16)
            nc.vector.memset(warm[:], 0.0)

            pss = [
                psum_pool.tile([P, NF], F32, tag="ps", name=f"ps0_{mt}")
                for mt in range(MT)
            ]
            for i in range(N_WARMUP_MM):
                nc.tensor.matmul(
                    pss[0][:],
                    warm[0:1, 0:P],
                    warm[0:1, :],
                    start=True,
                    stop=True,
                )

            bias_sb = const_pool.tile([P, OUT_F], F32)
            x_t = x_pool.tile([P, KF, M], F16)
            x8_t = x_pool.tile([P, C8, M], F8)
            w16s = {
                0: w_pool.tile([P, KF, NF], F16, tag="w", name="w16_0")
            }
            w8s = {
                0: w8_pool.tile([P, C8, NF], F8, tag="w8", name="w8_0")
            }

            def evict(ps, mt, nt, split=1):
                # ot = fp16(psum + bias)   (scale is folded into w on host)
                # split>1 pipelines add+store in slices to shorten the kernel
                # tail after the very last matmul.
                ot = out_pool.tile([P, NF], F16)
                nw = NF // split
                for s in range(split):
                    sl = slice(s * nw, (s + 1) * nw)
                    osl = slice(nt * NF + s * nw, nt * NF + (s + 1) * nw)
                    nc.vector.tensor_add(ot[:, sl], ps[:, sl], bias_sb[:, osl])
                    nc.sync.dma_start(out_view[:, mt, osl], ot[:, sl])

            def dr_matmuls(ps, w8_tile, mt):
                # fp8 DoubleRow: each matmul contracts 2 k-chunks (256) at
                # 2x MAC rate; accumulates into the open fp16 PSUM group.
                for g in range(G8):
                    nc.tensor.matmul(
                        ps[:],
                        x8_t[:, 2 * g : 2 * g + 2, mt * P : (mt + 1) * P],
                        w8_tile[:, 2 * g : 2 * g + 2, :],
                        start=False,
                        stop=(g == G8 - 1),
                        perf_mode=DR,
                    )

            # --- First n-tile: k-outer / m-inner across all 8 PSUM banks so
            # each arriving (x, w) k-chunk unlocks MT matmuls. x chunks have
            # the sync HWDGE ring to themselves; EVERYTHING else (w chunks,
            # then bias, then the nt-slabs, in FIFO order) shares the scalar
            # HWDGE ring, so round-robin between the two rings keeps x
            # delivery ahead of consumption.
            # x chunks have the sync HWDGE ring to themselves; EVERYTHING
            # else (w chunks, then bias, then the nt-slabs, in FIFO order)
            # shares the scalar HWDGE ring, so round-robin between the two
            # rings keeps x delivery ahead of consumption. The first chunks
            # are single k-slices so compute starts early; later chunks are
            # 4-slice (1MB) groups — small transfers only reach ~60% of DMA
            # line rate, which is what starved the tail of this phase.
            def xw_group(k0, klen):
                nc.scalar.dma_start(
                    w16s[0][:, k0 : k0 + klen, :],
                    w_view[0, :, k0 : k0 + klen, :],
                )
                nc.sync.dma_start(
                    x_t[:, k0 : k0 + klen, :], x_view[:, k0 : k0 + klen, :]
                )

            for k0 in range(0, 8):
                xw_group(k0, 1)
            for k0 in range(8, KF, 4):
                xw_group(k0, 4)

            for ko in range(KF):
                for mt in range(MT):
                    nc.tensor.matmul(
                        pss[mt][:],
                        x_t[:, ko, mt * P : (mt + 1) * P],
                        w16s[0][:, ko, :],
                        start=(ko == 0),
                        stop=False,
                    )

            # fp8 tail of the contraction for nt=0
            nc.scalar.dma_start(w8s[0][:], w8_view[0])
            nc.sync.dma_start(x8_t[:], x8_view[:])
            for mt in range(MT):
                dr_matmuls(pss[mt], w8s[0], mt)

            # bias queues on the scalar ring behind the w0 chunks: streams in
            # well before the first eviction needs it.
            nc.scalar.dma_start(bias_sb[:], bias_rep[:])

            for mt in range(MT):
                evict(pss[mt], mt, 0)

            # --- Remaining n-tiles: x is fully resident; weights arrive as
            # one 3.5MB slab (+0.25MB fp8 slab) per n-tile, double-buffered.
            for nt in range(1, NT):
                wn = w_pool.tile([P, KF, NF], F16, tag="w", name=f"w16_{nt}")
                nc.scalar.dma_start(wn[:], w_view[nt])
                w8n = w8_pool.tile([P, C8, NF], F8, tag="w8", name=f"w8_{nt}")
                nc.scalar.dma_start(w8n[:], w8_view[nt])
                for mt in range(MT):
                    ps = psum_pool.tile([P, NF], F32, tag="ps", name=f"ps{nt}_{mt}")
                    for ko in range(KF):
                        nc.tensor.matmul(
                            ps[:],
                            x_t[:, ko, mt * P : (mt + 1) * P],
                            wn[:, ko, :],
                            start=(ko == 0),
                            stop=False,
                        )
                    dr_matmuls(ps, w8n, mt)
                    last = nt == NT - 1 and mt == MT - 1
                    evict(ps, mt, nt, split=4 if last else 1)

    _split_sync_waits(nc)
    return nc


def kernel(x, weight_int8, scale, bias):
    global LAST_RESULTS
    x = np.asarray(x)
    weight_int8 = np.asarray(weight_int8)
    scale = np.asarray(scale)
    bias = np.asarray(bias)

    np_f8 = mybir.dt.np(F8)
    KSPLIT = KF * P  # contraction index where fp16 ends and fp8 begins

    x2d = x.reshape(M_TOTAL, IN_F)

    # w16[nt, p, ko, nf] = (w.T * scale)[ko*128+p, nt*512+nf], fp8 tail apart
    wT = weight_int8.T.astype(np.float32) * np.float32(scale)
    w16_dev = np.ascontiguousarray(
        wT[:KSPLIT]
        .astype(np.float16)
        .reshape(KF, P, NT, NF)
        .transpose(2, 1, 0, 3)
    ).reshape(NT * P, KF * NF)
    w8_dev = np.ascontiguousarray(
        wT[KSPLIT:]
        .astype(np_f8)
        .reshape(C8, P, NT, NF)
        .transpose(2, 1, 0, 3)
    ).reshape(NT * P, C8 * NF)

    bias_rep = np.ascontiguousarray(
        np.broadcast_to(bias.astype(np.float32, copy=False), (P, OUT_F))
    )

    in_maps = []
    for c in range(N_CORES):
        shard_t = x2d[c * M : (c + 1) * M].T       # [4096, 1024]
        x16_c = np.ascontiguousarray(
            shard_t[:KSPLIT].astype(np.float16).reshape(KF, P, M).transpose(1, 0, 2)
        ).reshape(P, KF * M)
        x8_c = np.ascontiguousarray(
            shard_t[KSPLIT:].astype(np_f8).reshape(C8, P, M).transpose(1, 0, 2)
        ).reshape(P, C8 * M)
        in_maps.append(
            {
                "x16": x16_c,
                "x8": x8_c,
                "w16": w16_dev,
                "w8": w8_dev,
                "bias_rep": bias_rep,
            }
        )

    nc = build_nc()
    trace = bool(int(os.environ.get("KERNEL_TRACE", "0")))
    res = run_bass_kernel_spmd(nc, in_maps, list(range(N_CORES)), trace=trace)
    LAST_RESULTS = res

    out = np.empty((M_TOTAL, OUT_F), dtype=np.float32)
    for c in range(N_CORES):
        out[c * M : (c + 1) * M] = res.results[c]["out"].astype(np.float32)
    return out.reshape(B, S, OUT_F)


# revision 31
# speedup vs baseline: 1.1746x; 1.1746x over previous
"""Trainium2 kernel for nn_CompressedLinearRANS: out = x @ (w_int8*scale).T + bias.

Strategy: data-parallel over the 8192 rows of x (1024 rows per core, full
weight replicated per core). Per-core matmul M=1024, K=4096, N=4096 on the
PE array: the first 3584 of the 4096 contraction elements run in fp16, the
last 512 run as fp8(e4m3) DoubleRow matmuls (2x MAC rate). The fp8 share is
sized so the total quantization error (~1.5e-2 max-rel) stays safely inside
the 2e-2 tolerance.

All dtype conversion happens on the host: x/w are rounded to fp16/e4m3 (with
the per-tensor scale folded into w), so the device does only DMA + matmul +
a single fused bias-add/downcast eviction per output tile.

Host-prepped inputs per core:
  x16  [128, 28*1024] fp16  (fp16 part of x^T shard, k-chunk-major [p, ko, m])
  x8   [128, 4*1024]  e4m3  (fp8 part)
  w16  [8*128, 28*512] fp16 ((w.T*scale) fp16 part, slab-major [nt, p, ko, nf])
  w8   [8*128, 4*512]  e4m3 (fp8 part)
  bias_rep [128, 4096] fp32 (bias broadcast to 128 partitions)
Output per core: out [1024, 4096] fp16 ([p, mt, n] over partitions).
"""

import os

import numpy as np

import concourse.bass as bass
import concourse.mybir as mybir
import concourse.tile as tile
from concourse.bass_utils import run_bass_kernel_spmd

# ---------------------------------------------------------------------------
# Workaround for this walrus build: it rejects instructions carrying more
# than one sync wait command ("Too many sync wait commands",
# CoreV*GenImpl::setupSyncWait). After Tile lowering, split any instruction
# with >1 sem wait: keep the last wait on the instruction and hoist the rest
# onto same-engine NoOps inserted immediately before it (engines execute
# their stream in order, so this is semantics-preserving).
# ---------------------------------------------------------------------------


def _split_sync_waits(nc, max_waits=1):
    for fn in nc.m.functions:
        for bb in fn.blocks:
            out = []
            changed = False
            for inst in bb.instructions:
                si = inst.sync_info
                waits = list(si.on_wait) if si is not None and si.on_wait else []
                if len(waits) > max_waits:
                    changed = True
                    for w in waits[:-max_waits]:
                        nop = mybir.InstNoOp(
                            name=nc.get_next_instruction_name(),
                            sync_info=mybir.SyncInfo(on_wait=[w], on_update=[]),
                            bass_nofuse=True,
                            engine=inst.engine,
                        )
                        nc.register_instruction(nop)
                        out.append(nop)
                    inst.sync_info = mybir.SyncInfo(
                        on_wait=waits[-max_waits:],
                        on_update=list(si.on_update or []),
                    )
                out.append(inst)
            if changed:
                bb.instructions = out


# ---------------------------------------------------------------------------

P = 128
N_CORES = 8

OUT_F, IN_F = 4096, 4096
B, S = 4, 2048
M_TOTAL = B * S           # 8192 rows
M = M_TOTAL // N_CORES    # 1024 rows per core

KO = IN_F // P            # 32 k-chunks of 128
C8 = 4                    # k-chunks computed in fp8 DoubleRow
KF = KO - C8              # k-chunks computed in fp16
G8 = C8 // 2              # DoubleRow matmuls per output tile (2 chunks each)
MT = M // P               # 8 m-tiles of 128
NF = 512                  # moving free dim / PSUM bank width (fp32)
NT = OUT_F // NF          # 8 n-tiles

F32 = mybir.dt.float32
F16 = mybir.dt.float16
F8 = mybir.dt.float8e4
DR = mybir.MatmulPerfMode.DoubleRow

N_WARMUP_MM = 8           # PE clock warmup during the DMA lead-in (N=512 each)

# exposed for test.py
LAST_RESULTS = None


def build_nc(trn_type=None):
    """Build the per-core Bass module."""
    nc = bass.Bass() if trn_type is None else bass.Bass(trn_type)
    x16 = nc.dram_tensor("x16", [P, KF * M], F16, kind="ExternalInput")
    x8 = nc.dram_tensor("x8", [P, C8 * M], F8, kind="ExternalInput")
    w16 = nc.dram_tensor("w16", [NT * P, KF * NF], F16, kind="ExternalInput")
    w8 = nc.dram_tensor("w8", [NT * P, C8 * NF], F8, kind="ExternalInput")
    bias_rep = nc.dram_tensor("bias_rep", [P, OUT_F], F32, kind="ExternalInput")
    out = nc.dram_tensor("out", [M, OUT_F], F16, kind="ExternalOutput")

    x_view = x16.rearrange("p (ko m) -> p ko m", ko=KF)        # [P, KF, M]
    x8_view = x8.rearrange("p (c m) -> p c m", c=C8)           # [P, C8, M]
    w_view = w16.rearrange("(nt p) (ko nf) -> nt p ko nf", nt=NT, ko=KF)
    w8_view = w8.rearrange("(nt p) (c nf) -> nt p c nf", nt=NT, c=C8)
    out_view = out.rearrange("(mt p) n -> p mt n", p=P)        # [P, MT, N]

    with tile.TileContext(nc) as tc:
        with (
            tc.tile_pool(name="const", bufs=1) as const_pool,
            tc.tile_pool(name="xsb", bufs=1) as x_pool,
            tc.tile_pool(name="wsb", bufs=2) as w_pool,
            tc.tile_pool(name="w8sb", bufs=2) as w8_pool,
            tc.tile_pool(name="outsb", bufs=6) as out_pool,
            tc.tile_pool(name="warm", bufs=1) as warm_pool,
            tc.tile_pool(name="psum", bufs=8, space="PSUM") as psum_pool,
        ):
            # --- PE clock warmup: tiny self-contained matmuls on zeroed SBUF
            # during the initial DMA wait, so the HAM un-throttles before the
            # first real matmul. They write (start=True) into psum tiles that
            # the real accumulation groups reset anyway.
            warm = warm_pool.tile([2, NF], F16)
            nc.vector.memset(warm[:], 0.0)

            pss = [
                psum_pool.tile([P, NF], F32, tag="ps", name=f"ps0_{mt}")
                for mt in range(MT)
            ]
            for i in range(N_WARMUP_MM):
                nc.tensor.matmul(
                    pss[0][:],
                    warm[0:1, 0:P],
                    warm[0:1, :],
                    start=True,
                    stop=True,
                )

            bias_sb = const_pool.tile([P, OUT_F], F32)
            x_t = x_pool.tile([P, KF, M], F16)
            x8_t = x_pool.tile([P, C8, M], F8)
            w16s = {
                0: w_pool.tile([P, KF, NF], F16, tag="w", name="w16_0")
            }
            w8s = {
                0: w8_pool.tile([P, C8, NF], F8, tag="w8", name="w8_0")
            }

            def evict(ps, mt, nt, split=1):
                # ot = fp16(psum + bias)   (scale is folded into w on host)
                # split>1 pipelines add+store in slices to shorten the kernel
                # tail after the very last matmul.
                ot = out_pool.tile([P, NF], F16)
                nw = NF // split
                for s in range(split):
                    sl = slice(s * nw, (s + 1) * nw)
                    osl = slice(nt * NF + s * nw, nt * NF + (s + 1) * nw)
                    nc.vector.tensor_add(ot[:, sl], ps[:, sl], bias_sb[:, osl])
                    nc.sync.dma_start(out_view[:, mt, osl], ot[:, sl])

            def dr_matmuls(ps, w8_tile, mt):
                # fp8 DoubleRow: each matmul contracts 2 k-chunks (256) at
                # 2x MAC rate; accumulates into the open fp16 PSUM group.
                for g in range(G8):
                    nc.tensor.matmul(
                        ps[:],
                        x8_t[:, 2 * g : 2 * g + 2, mt * P : (mt + 1) * P],
                        w8_tile[:, 2 * g : 2 * g + 2, :],
                        start=False,
                        stop=(g == G8 - 1),
                        perf_mode=DR,
                    )

            # --- First n-tile: k-outer / m-inner across all 8 PSUM banks so
            # each arriving (x, w) k-chunk unlocks MT matmuls. x chunks have
            # the sync HWDGE ring to themselves; EVERYTHING else (w chunks,
            # then bias, then the nt-slabs, in FIFO order) shares the scalar
            # HWDGE ring, so round-robin between the two rings keeps x
            # delivery ahead of consumption.
            # x chunks have the sync HWDGE ring to themselves; EVERYTHING
            # else (w chunks, then bias, then the nt-slabs, in FIFO order)
            # shares the scalar HWDGE ring, so round-robin between the two
            # rings keeps x delivery ahead of consumption.
            for ko in range(KF):
                nc.scalar.dma_start(
                    w16s[0][:, ko, :], w_view[0, :, ko, :]
                )
                nc.sync.dma_start(x_t[:, ko, :], x_view[:, ko, :])
                for mt in range(MT):
                    nc.tensor.matmul(
                        pss[mt][:],
                        x_t[:, ko, mt * P : (mt + 1) * P],
                        w16s[0][:, ko, :],
                        start=(ko == 0),
                        stop=False,
                    )

            # fp8 tail of the contraction for nt=0
            nc.scalar.dma_start(w8s[0][:], w8_view[0])
            nc.sync.dma_start(x8_t[:], x8_view[:])
            for mt in range(MT):
                dr_matmuls(pss[mt], w8s[0], mt)

            # bias queues on the scalar ring behind the w0 chunks: streams in
            # well before the first eviction needs it.
            nc.scalar.dma_start(bias_sb[:], bias_rep[:])

            for mt in range(MT):
                evict(pss[mt], mt, 0)

            # --- Remaining n-tiles: x is fully resident; weights arrive as
            # one 3.5MB slab (+0.25MB fp8 slab) per n-tile, double-buffered.
            for nt in range(1, NT):
                wn = w_pool.tile([P, KF, NF], F16, tag="w", name=f"w16_{nt}")
                nc.scalar.dma_start(wn[:], w_view[nt])
                w8n = w8_pool.tile([P, C8, NF], F8, tag="w8", name=f"w8_{nt}")
                nc.scalar.dma_start(w8n[:], w8_view[nt])
                for mt in range(MT):
                    ps = psum_pool.tile([P, NF], F32, tag="ps", name=f"ps{nt}_{mt}")
                    for ko in range(KF):
                        nc.tensor.matmul(
                            ps[:],
                            x_t[:, ko, mt * P : (mt + 1) * P],
                            wn[:, ko, :],
                            start=(ko == 0),
                            stop=False,
                        )
                    dr_matmuls(ps, w8n, mt)
                    last = nt == NT - 1 and mt == MT - 1
                    evict(ps, mt, nt, split=4 if last else 1)

    _split_sync_waits(nc)
    return nc


def kernel(x, weight_int8, scale, bias):
    global LAST_RESULTS
    x = np.asarray(x)
    weight_int8 = np.asarray(weight_int8)
    scale = np.asarray(scale)
    bias = np.asarray(bias)

    np_f8 = mybir.dt.np(F8)
    KSPLIT = KF * P  # contraction index where fp16 ends and fp8 begins

    x2d = x.reshape(M_TOTAL, IN_F)

    # w16[nt, p, ko, nf] = (w.T * scale)[ko*128+p, nt*512+nf], fp8 tail apart
    wT = weight_int8.T.astype(np.float32) * np.float32(scale)
    w16_dev = np.ascontiguousarray(
        wT[:KSPLIT]
        .astype(np.float16)
        .reshape(KF, P, NT, NF)
        .transpose(2, 1, 0, 3)
    ).reshape(NT * P, KF * NF)
    w8_dev = np.ascontiguousarray(
        wT[KSPLIT:]
        .astype(np_f8)
        .reshape(C8, P, NT, NF)
        .transpose(2, 1, 0, 3)
    ).reshape(NT * P, C8 * NF)

    bias_rep = np.ascontiguousarray(
        np.broadcast_to(bias.astype(np.float32, copy=False), (P, OUT_F))
    )

    in_maps = []
    for c in range(N_CORES):
        shard_t = x2d[c * M : (c + 1) * M].T       # [4096, 1024]
        x16_c = np.ascontiguousarray(
            shard_t[:KSPLIT].astype(np.float16).reshape(KF, P, M).transpose(1, 0, 2)
        ).reshape(P, KF * M)
        x8_c = np.ascontiguousarray(
            shard_t[KSPLIT:].astype(np_f8).reshape(C8, P, M).transpose(1, 0, 2)
        ).reshape(P, C8 * M)
        in_maps.append(
            {
                "x16": x16_c,
                "x8": x8_c,
                "w16": w16_dev,
                "w8": w8_dev,
                "bias_rep": bias_rep,
            }
        )

    nc = build_nc()
    trace = bool(int(os.environ.get("KERNEL_TRACE", "0")))
    res = run_bass_kernel_spmd(nc, in_maps, list(range(N_CORES)), trace=trace)
    LAST_RESULTS = res

    out = np.empty((M_TOTAL, OUT_F), dtype=np.float32)
    for c in range(N_CORES):
        out[c * M : (c + 1) * M] = res.results[c]["out"].astype(np.float32)
    return out.reshape(B, S, OUT_F)


# revision 35
# speedup vs baseline: 1.1841x; 1.0081x over previous
"""Trainium2 kernel for nn_CompressedLinearRANS: out = x @ (w_int8*scale).T + bias.

Strategy: data-parallel over the 8192 rows of x (1024 rows per core, full
weight replicated per core). Per-core matmul M=1024, K=4096, N=4096 on the
PE array: the first 3584 of the 4096 contraction elements run in fp16, the
last 512 run as fp8(e4m3) DoubleRow matmuls (2x MAC rate). The fp8 share is
sized so the total quantization error (~1.5e-2 max-rel) stays safely inside
the 2e-2 tolerance.

All dtype conversion happens on the host: x/w are rounded to fp16/e4m3 (with
the per-tensor scale folded into w), so the device does only DMA + matmul +
a single fused bias-add/downcast eviction per output tile.

Host-prepped inputs per core:
  x16  [128, 28*1024] fp16  (fp16 part of x^T shard, k-chunk-major [p, ko, m])
  x8   [128, 4*1024]  e4m3  (fp8 part)
  w16  [8*128, 28*512] fp16 ((w.T*scale) fp16 part, slab-major [nt, p, ko, nf])
  w8   [8*128, 4*512]  e4m3 (fp8 part)
  bias_rep [128, 4096] fp32 (bias broadcast to 128 partitions)
Output per core: out [1024, 4096] fp16 ([p, mt, n] over partitions).
"""

import os

import numpy as np

import concourse.bass as bass
import concourse.mybir as mybir
import concourse.tile as tile
from concourse.bass_utils import run_bass_kernel_spmd

# ---------------------------------------------------------------------------
# Workaround for this walrus build: it rejects instructions carrying more
# than one sync wait command ("Too many sync wait commands",
# CoreV*GenImpl::setupSyncWait). After Tile lowering, split any instruction
# with >1 sem wait: keep the last wait on the instruction and hoist the rest
# onto same-engine NoOps inserted immediately before it (engines execute
# their stream in order, so this is semantics-preserving).
# ---------------------------------------------------------------------------


def _split_sync_waits(nc, max_waits=1):
    for fn in nc.m.functions:
        for bb in fn.blocks:
            out = []
            changed = False
            for inst in bb.instructions:
                si = inst.sync_info
                waits = list(si.on_wait) if si is not None and si.on_wait else []
                if len(waits) > max_waits:
                    changed = True
                    for w in waits[:-max_waits]:
                        nop = mybir.InstNoOp(
                            name=nc.get_next_instruction_name(),
                            sync_info=mybir.SyncInfo(on_wait=[w], on_update=[]),
                            bass_nofuse=True,
                            engine=inst.engine,
                        )
                        nc.register_instruction(nop)
                        out.append(nop)
                    inst.sync_info = mybir.SyncInfo(
                        on_wait=waits[-max_waits:],
                        on_update=list(si.on_update or []),
                    )
                out.append(inst)
            if changed:
                bb.instructions = out


# ---------------------------------------------------------------------------

P = 128
N_CORES = 8

OUT_F, IN_F = 4096, 4096
B, S = 4, 2048
M_TOTAL = B * S           # 8192 rows
M = M_TOTAL // N_CORES    # 1024 rows per core

KO = IN_F // P            # 32 k-chunks of 128
C8 = 4                    # k-chunks computed in fp8 DoubleRow
KF = KO - C8              # k-chunks computed in fp16
G8 = C8 // 2              # DoubleRow matmuls per output tile (2 chunks each)
MT = M // P               # 8 m-tiles of 128
NF = 512                  # moving free dim / PSUM bank width (fp32)
NT = OUT_F // NF          # 8 n-tiles

F32 = mybir.dt.float32
F16 = mybir.dt.float16
F8 = mybir.dt.float8e4
DR = mybir.MatmulPerfMode.DoubleRow

N_WARMUP_MM = 64          # PE clock warmup during the DMA lead-in

# exposed for test.py
LAST_RESULTS = None


def build_nc(trn_type=None):
    """Build the per-core Bass module."""
    nc = bass.Bass() if trn_type is None else bass.Bass(trn_type)
    x16 = nc.dram_tensor("x16", [P, KF * M], F16, kind="ExternalInput")
    x8 = nc.dram_tensor("x8", [P, C8 * M], F8, kind="ExternalInput")
    w16 = nc.dram_tensor("w16", [NT * P, KF * NF], F16, kind="ExternalInput")
    w8 = nc.dram_tensor("w8", [NT * P, C8 * NF], F8, kind="ExternalInput")
    bias_rep = nc.dram_tensor("bias_rep", [P, OUT_F], F32, kind="ExternalInput")
    out = nc.dram_tensor("out", [M, OUT_F], F16, kind="ExternalOutput")

    x_view = x16.rearrange("p (ko m) -> p ko m", ko=KF)        # [P, KF, M]
    x8_view = x8.rearrange("p (c m) -> p c m", c=C8)           # [P, C8, M]
    w_view = w16.rearrange("(nt p) (ko nf) -> nt p ko nf", nt=NT, ko=KF)
    w8_view = w8.rearrange("(nt p) (c nf) -> nt p c nf", nt=NT, c=C8)
    out_view = out.rearrange("(mt p) n -> p mt n", p=P)        # [P, MT, N]

    with tile.TileContext(nc) as tc:
        with (
            tc.tile_pool(name="const", bufs=1) as const_pool,
            tc.tile_pool(name="xsb", bufs=1) as x_pool,
            tc.tile_pool(name="wsb", bufs=2) as w_pool,
            tc.tile_pool(name="w8sb", bufs=2) as w8_pool,
            tc.tile_pool(name="outsb", bufs=6) as out_pool,
            tc.tile_pool(name="warm", bufs=1) as warm_pool,
            tc.tile_pool(name="psum", bufs=8, space="PSUM") as psum_pool,
        ):
            # --- PE clock warmup: tiny self-contained matmuls on zeroed SBUF
            # during the initial DMA wait, so the HAM un-throttles before the
            # first real matmul. They write (start=True) into psum tiles that
            # the real accumulation groups reset anyway.
            warm = warm_pool.tile([2, P], F16)
            nc.vector.memset(warm[:], 0.0)

            pss = [
                psum_pool.tile([P, NF], F32, tag="ps", name=f"ps0_{mt}")
                for mt in range(MT)
            ]
            for i in range(N_WARMUP_MM):
                nc.tensor.matmul(
                    pss[0][:, 0:64],
                    warm[0:1, :],
                    warm[0:1, 0:64],
                    start=True,
                    stop=True,
                )

            bias_sb = const_pool.tile([P, OUT_F], F32)
            x_t = x_pool.tile([P, KF, M], F16)
            x8_t = x_pool.tile([P, C8, M], F8)
            w16s = {
                0: w_pool.tile([P, KF, NF], F16, tag="w", name="w16_0")
            }
            w8s = {
                0: w8_pool.tile([P, C8, NF], F8, tag="w8", name="w8_0")
            }

            def evict(ps, mt, nt, split=1):
                # ot = fp16(psum + bias)   (scale is folded into w on host)
                # split>1 pipelines add+store in slices to shorten the kernel
                # tail after the very last matmul.
                ot = out_pool.tile([P, NF], F16)
                nw = NF // split
                for s in range(split):
                    sl = slice(s * nw, (s + 1) * nw)
                    osl = slice(nt * NF + s * nw, nt * NF + (s + 1) * nw)
                    nc.vector.tensor_add(ot[:, sl], ps[:, sl], bias_sb[:, osl])
                    nc.sync.dma_start(out_view[:, mt, osl], ot[:, sl])

            def dr_matmuls(ps, w8_tile, mt):
                # fp8 DoubleRow: each matmul contracts 2 k-chunks (256) at
                # 2x MAC rate; accumulates into the open fp16 PSUM group.
                for g in range(G8):
                    nc.tensor.matmul(
                        ps[:],
                        x8_t[:, 2 * g : 2 * g + 2, mt * P : (mt + 1) * P],
                        w8_tile[:, 2 * g : 2 * g + 2, :],
                        start=False,
                        stop=(g == G8 - 1),
                        perf_mode=DR,
                    )

            # --- First n-tile: k-outer / m-inner across all 8 PSUM banks so
            # each arriving (x, w) k-chunk unlocks MT matmuls. x chunks have
            # the sync HWDGE ring to themselves; EVERYTHING else (w chunks,
            # then bias, then the nt-slabs, in FIFO order) shares the scalar
            # HWDGE ring, so round-robin between the two rings keeps x
            # delivery ahead of consumption.
            # x chunks have the sync HWDGE ring to themselves; EVERYTHING
            # else (w chunks, then bias, then the nt-slabs, in FIFO order)
            # shares the scalar HWDGE ring, so round-robin between the two
            # rings keeps x delivery ahead of consumption.
            for ko in range(KF):
                nc.scalar.dma_start(
                    w16s[0][:, ko, :], w_view[0, :, ko, :]
                )
                # alternate x chunks between the sync HWDGE ring and the
                # gpsimd SWDGE ring so x delivery outpaces consumption
                x_eng = nc.sync if ko % 2 == 0 else nc.gpsimd
                x_eng.dma_start(x_t[:, ko, :], x_view[:, ko, :])
                for mt in range(MT):
                    nc.tensor.matmul(
                        pss[mt][:],
                        x_t[:, ko, mt * P : (mt + 1) * P],
                        w16s[0][:, ko, :],
                        start=(ko == 0),
                        stop=False,
                    )

            # fp8 tail of the contraction for nt=0
            nc.scalar.dma_start(w8s[0][:], w8_view[0])
            nc.gpsimd.dma_start(x8_t[:], x8_view[:])
            for mt in range(MT):
                dr_matmuls(pss[mt], w8s[0], mt)

            # bias queues on the scalar ring behind the w0 chunks: streams in
            # well before the first eviction needs it.
            nc.scalar.dma_start(bias_sb[:], bias_rep[:])

            for mt in range(MT):
                evict(pss[mt], mt, 0)

            # --- Remaining n-tiles: x is fully resident; weights arrive as
            # one 3.5MB slab (+0.25MB fp8 slab) per n-tile, double-buffered.
            for nt in range(1, NT):
                wn = w_pool.tile([P, KF, NF], F16, tag="w", name=f"w16_{nt}")
                nc.scalar.dma_start(wn[:], w_view[nt])
                w8n = w8_pool.tile([P, C8, NF], F8, tag="w8", name=f"w8_{nt}")
                nc.scalar.dma_start(w8n[:], w8_view[nt])
                for mt in range(MT):
                    ps = psum_pool.tile([P, NF], F32, tag="ps", name=f"ps{nt}_{mt}")
                    for ko in range(KF):
                        nc.tensor.matmul(
                            ps[:],
                            x_t[:, ko, mt * P : (mt + 1) * P],
                            wn[:, ko, :],
                            start=(ko == 0),
                            stop=False,
                        )
                    dr_matmuls(ps, w8n, mt)
                    last = nt == NT - 1 and mt == MT - 1
                    evict(ps, mt, nt, split=4 if last else 1)

    _split_sync_waits(nc)
    return nc


def kernel(x, weight_int8, scale, bias):
    global LAST_RESULTS
    x = np.asarray(x)
    weight_int8 = np.asarray(weight_int8)
    scale = np.asarray(scale)
    bias = np.asarray(bias)

    np_f8 = mybir.dt.np(F8)
    KSPLIT = KF * P  # contraction index where fp16 ends and fp8 begins

    x2d = x.reshape(M_TOTAL, IN_F)

    # w16[nt, p, ko, nf] = (w.T * scale)[ko*128+p, nt*512+nf], fp8 tail apart
    wT = weight_int8.T.astype(np.float32) * np.float32(scale)
    w16_dev = np.ascontiguousarray(
        wT[:KSPLIT]
        .astype(np.float16)
        .reshape(KF, P, NT, NF)
        .transpose(2, 1, 0, 3)
    ).reshape(NT * P, KF * NF)
    w8_dev = np.ascontiguousarray(
        wT[KSPLIT:]
        .astype(np_f8)
        .reshape(C8, P, NT, NF)
        .transpose(2, 1, 0, 3)
    ).reshape(NT * P, C8 * NF)

    bias_rep = np.ascontiguousarray(
        np.broadcast_to(bias.astype(np.float32, copy=False), (P, OUT_F))
    )

    in_maps = []
    for c in range(N_CORES):
        shard_t = x2d[c * M : (c + 1) * M].T       # [4096, 1024]
        x16_c = np.ascontiguousarray(
            shard_t[:KSPLIT].astype(np.float16).reshape(KF, P, M).transpose(1, 0, 2)
        ).reshape(P, KF * M)
        x8_c = np.ascontiguousarray(
            shard_t[KSPLIT:].astype(np_f8).reshape(C8, P, M).transpose(1, 0, 2)
        ).reshape(P, C8 * M)
        in_maps.append(
            {
                "x16": x16_c,
                "x8": x8_c,
                "w16": w16_dev,
                "w8": w8_dev,
                "bias_rep": bias_rep,
            }
        )

    nc = build_nc()
    trace = bool(int(os.environ.get("KERNEL_TRACE", "0")))
    res = run_bass_kernel_spmd(nc, in_maps, list(range(N_CORES)), trace=trace)
    LAST_RESULTS = res

    out = np.empty((M_TOTAL, OUT_F), dtype=np.float32)
    for c in range(N_CORES):
        out[c * M : (c + 1) * M] = res.results[c]["out"].astype(np.float32)
    return out.reshape(B, S, OUT_F)
